# revision 2
# baseline (speedup 1.0000x reference)
"""Causal multi-head self-attention (B=4, S=2048, D=1024, H=16, RoPE) on 8 NeuronCores.

Sharding: core c handles batch b = c // 2 and heads [8*(c%2), 8*(c%2)+8).
Each core computes its 8 heads' attention plus the partial W_O projection
(columns owned by its heads); host sums the two partials per batch.

v2: single causal-pipelined loop over s-blocks — for each 512-wide block sb:
  phase1(sb): QKV projections + RoPE for block sb
  attn(qb=sb): all 8 heads attend queries sb against keys 0..sb (available!)
  phase3(sb): W_O partial projection for block sb
This keeps the PE warm (no multi-us idle gaps -> no HAM re-throttle) and
overlaps the ACT-bound softmax with PE-bound projection work.
Softmax denominators: ones-column in V -> pv row 64; reciprocal_approx_fast;
gpsimd partition_broadcast; DVE multiply straight out of PSUM (no un copies,
no broadcast matmuls). Causal mask: DVE 0/1-mask multiply on exp output
(replaces PE bias matmuls).
"""
import math
import os
from contextlib import ExitStack, nullcontext

import numpy as np

B, S, D, H, DK = 4, 2048, 1024, 16, 64
HP = 8            # heads per core
NCORES = 8
THETA = 10000.0
SB = 512          # s-block width
NSB = S // SB     # 4
NIC = D // 128    # 8 in-chunks
NDC = (HP * DK) // 128   # 4 dk-chunks (local) = head pairs
NKC = S // 128    # 16 k-chunks

_BUILD_CACHE = {}


def _build(repeat=1):
    import concourse.tile as tile
    from concourse import bacc, mybir

    F32 = mybir.dt.float32
    BF16 = mybir.dt.bfloat16
    EXP = mybir.ActivationFunctionType.Exp
    MULT = mybir.AluOpType.mult
    ADD = mybir.AluOpType.add

    nc = bacc.Bacc("TRN2", target_bir_lowering=False, debug=False,
                   num_devices=NCORES)
    xT_d = nc.declare_dram_parameter("xT", [D, S], BF16, isOutput=False)
    wqT_d = nc.declare_dram_parameter("wqT", [D, HP * DK], BF16, isOutput=False)
    wkT_d = nc.declare_dram_parameter("wkT", [D, HP * DK], BF16, isOutput=False)
    wvT_d = nc.declare_dram_parameter("wvT", [D, HP * DK], BF16, isOutput=False)
    woT_d = nc.declare_dram_parameter("woT", [HP * DK, D], BF16, isOutput=False)
    cos_d = nc.declare_dram_parameter("cosR", [128, S], F32, isOutput=False)
    sin_d = nc.declare_dram_parameter("sinR", [128, S], F32, isOutput=False)
    swp_d = nc.declare_dram_parameter("swp", [128, 128], BF16, isOutput=False)
    mask_d = nc.declare_dram_parameter("masken", [128, 2 * 128], BF16, isOutput=False)
    onesb_d = nc.declare_dram_parameter("onesb", [128, 128], BF16, isOutput=False)
    out_d = nc.declare_dram_parameter("out", [S, D], BF16, isOutput=True)

    xT_r = xT_d.rearrange("(ic p) (sb s) -> p ic sb s", p=128, s=SB)
    out_r = out_d.rearrange("(sc p) o -> p sc o", p=128)

    with tile.TileContext(nc) as tc, ExitStack() as octx:
        # long-lived tensors
        glob = octx.enter_context(tc.tile_pool(name="glob", bufs=1))
        QT = glob.tile([128, NDC, S], BF16, tag="QT", name="QT")
        KT = glob.tile([128, NDC, S], BF16, tag="KT", name="KT")
        V = glob.tile([128, NKC, HP, DK + 1], BF16, tag="V", name="V")
        AO = glob.tile([128, NDC, S], BF16, tag="AO", name="AO")
        cosR = glob.tile([128, S], F32, tag="cosR", name="cosR")
        sinR = glob.tile([128, S], F32, tag="sinR", name="sinR")
        swp = glob.tile([128, 128], BF16, tag="swp", name="swp")
        maskt = glob.tile([128, 2, 128], BF16, tag="maskt", name="maskt")
        wq_sb = glob.tile([128, NIC, HP * DK], BF16, tag="wq", name="wq_sb")
        wk_sb = glob.tile([128, NIC, HP * DK], BF16, tag="wk", name="wk_sb")
        wv_sb = glob.tile([128, NIC, HP * DK], BF16, tag="wv", name="wv_sb")
        wo_sb = glob.tile([128, NDC, D], BF16, tag="wo", name="wo_sb")
        nc.sync.dma_start(cosR[:], cos_d[:])
        nc.sync.dma_start(sinR[:], sin_d[:])
        nc.sync.dma_start(swp[:], swp_d[:])
        nc.sync.dma_start(maskt[:], mask_d.rearrange("p (a q) -> p a q", a=2))
        nc.sync.dma_start(wq_sb[:], wqT_d.rearrange("(ic p) m -> p ic m", p=128))
        nc.sync.dma_start(wk_sb[:], wkT_d.rearrange("(ic p) m -> p ic m", p=128))
        nc.sync.dma_start(wv_sb[:], wvT_d.rearrange("(ic p) m -> p ic m", p=128))
        nc.sync.dma_start(wo_sb[:], woT_d.rearrange("(c p) o -> p c o", p=128))
        nc.sync.dma_start(
            V[:, :, :, DK:DK + 1],
            onesb_d.rearrange("p (a b c) -> p a b c", a=NKC, b=HP))

        xpool = octx.enter_context(tc.tile_pool(name="xpool", bufs=2))
        rpool = octx.enter_context(tc.tile_pool(name="rope", bufs=3))
        epool = octx.enter_context(tc.tile_pool(name="epool", bufs=int(os.environ.get("ETBUFS", "6"))))
        npool = octx.enter_context(tc.tile_pool(name="npool", bufs=2))
        opool = octx.enter_context(tc.tile_pool(name="opool", bufs=2))
        gps = octx.enter_context(tc.tile_pool(name="gps", bufs=int(os.environ.get("GBUFS", "2")), space="PSUM"))
        sps = octx.enter_context(tc.tile_pool(name="sps", bufs=2, space="PSUM"))
        pvps = octx.enter_context(tc.tile_pool(name="pvps", bufs=1, space="PSUM"))

        loop_cm = tc.For_i(0, repeat, 1) if repeat > 1 else nullcontext()
        with loop_cm:
            for sb in range(NSB):
                ssl = slice(sb * SB, (sb + 1) * SB)
                # ---------------- phase 1: projections + RoPE for block sb ----
                x_sb = xpool.tile([128, NIC, SB], BF16, tag="x", name=f"x_{sb}")
                nc.sync.dma_start(x_sb[:], xT_r[:, :, sb, :])
                for w_sb, OT in ((wq_sb, QT), (wk_sb, KT)):
                    for c in range(NDC):
                        ps = gps.tile([128, SB], F32, tag="g", name=f"ps_{sb}_{c}")
                        for ic in range(NIC):
                            nc.tensor.matmul(
                                ps[:], w_sb[:, ic, c * 128:(c + 1) * 128],
                                x_sb[:, ic, :],
                                start=(ic == 0), stop=(ic == NIC - 1))
                        t1 = rpool.tile([128, SB], F32, tag="t1", name="t1")
                        nc.vector.tensor_tensor(t1[:], ps[:], cosR[:, ssl], MULT)
                        t2 = rpool.tile([128, SB], BF16, tag="t2", name="t2")
                        nc.vector.tensor_tensor(t2[:], ps[:], sinR[:, ssl], MULT)
                        t2s = gps.tile([128, SB], F32, tag="g", name="t2s")
                        nc.tensor.matmul(t2s[:], swp[:], t2[:],
                                         start=True, stop=True)
                        nc.vector.tensor_tensor(OT[:, c, ssl], t1[:], t2s[:], ADD)
                # V (natural layout), psum partition dim = s-chunk
                for sc4 in range(SB // 128):
                    sc = sb * 4 + sc4
                    ps = gps.tile([128, HP * DK], F32, tag="g", name=f"psv_{sc}")
                    for ic in range(NIC):
                        nc.tensor.matmul(
                            ps[:], x_sb[:, ic, sc4 * 128:(sc4 + 1) * 128],
                            wv_sb[:, ic, :],
                            start=(ic == 0), stop=(ic == NIC - 1))
                    nc.any.tensor_copy(
                        V[:, sc, :, 0:DK],
                        ps.rearrange("p (h v) -> p h v", h=HP))

                # ---------------- attention: queries block sb ----------------
                qb = sb
                nch = 4 * qb + 4
                qsl = slice(qb * SB, (qb + 1) * SB)
                for hp in range(NDC):
                    pv_a = pvps.tile([DK + 1, SB], F32, tag="pv_a", name=f"pva_{hp}_{qb}")
                    pv_b = pvps.tile([DK + 1, SB], F32, tag="pv_b", name=f"pvb_{hp}_{qb}")
                    for kc in range(nch):
                        ksl = slice(kc * 128, (kc + 1) * 128)
                        j = kc - 4 * qb
                        c0 = 128 * max(j, 0)
                        qsl0 = slice(qb * SB + c0, (qb + 1) * SB)
                        ps = sps.tile([128, 2, SB], F32, tag="s",
                                      name=f"pss_{hp}_{qb}_{kc}")
                        nc.tensor.matmul(
                            ps[:, 0, c0:], KT[0:64, hp, ksl], QT[0:64, hp, qsl0],
                            start=True, stop=True, tile_position=(0, 0))
                        nc.tensor.matmul(
                            ps[:, 1, c0:], KT[64:128, hp, ksl], QT[64:128, hp, qsl0],
                            start=True, stop=True, tile_position=(64, 0))
                        et = epool.tile([128, 2, SB], BF16, tag="et",
                                        name=f"et_{hp}_{qb}_{kc}")
                        nc.scalar.activation(et[:, :, c0:], ps[:, :, c0:],
                                             EXP, scale=1.0 / math.sqrt(DK))
                        if j >= 0:  # causal mask: 0/1 multiply on the diag block
                            nc.vector.tensor_tensor(
                                et[:, :, c0:c0 + 128], et[:, :, c0:c0 + 128],
                                maskt[:], MULT)
                        for x_, pv in ((0, pv_a), (1, pv_b)):
                            nc.tensor.matmul(
                                pv[:, c0:], V[:, kc, 2 * hp + x_, :],
                                et[:, x_, c0:],
                                start=(kc == 0), stop=(kc == nch - 1))
                    # normalize tail: recip of denominator row, broadcast, scale
                    dta = npool.tile([1, SB], F32, tag="dta", name="dta")
                    dtb = npool.tile([1, SB], F32, tag="dtb", name="dtb")
                    nc.vector.tensor_copy(dta[:], pv_a[DK:DK + 1, :])
                    nc.vector.tensor_copy(dtb[:], pv_b[DK:DK + 1, :])
                    ra = npool.tile([1, SB], F32, tag="ra", name="ra")
                    rb = npool.tile([1, SB], F32, tag="rb", name="rb")
                    nc.vector.reciprocal_approx_fast(out=ra[:], in_=dta[:])
                    nc.vector.reciprocal_approx_fast(out=rb[:], in_=dtb[:])
                    rbca = npool.tile([64, SB], F32, tag="rbca", name="rbca")
                    rbcb = npool.tile([64, SB], F32, tag="rbcb", name="rbcb")
                    nc.gpsimd.partition_broadcast(rbca[:], ra[:])
                    nc.gpsimd.partition_broadcast(rbcb[:], rb[:])
                    nc.vector.tensor_tensor(AO[0:64, hp, qsl], pv_a[0:DK, :],
                                            rbca[:], MULT)
                    nc.vector.tensor_tensor(AO[64:128, hp, qsl], pv_b[0:DK, :],
                                            rbcb[:], MULT)

                # ---------------- phase 3: W_O partials for block sb ----------
                for sc4 in range(SB // 128):
                    sc = sb * 4 + sc4
                    o_sb = opool.tile([128, D], BF16, tag="o", name=f"o_{sc}")
                    for ob in range(2):
                        ps = gps.tile([128, SB], F32, tag="g", name=f"ps3_{sc}_{ob}")
                        for c in range(NDC):
                            nc.tensor.matmul(
                                ps[:], AO[:, c, sc * 128:(sc + 1) * 128],
                                wo_sb[:, c, ob * SB:(ob + 1) * SB],
                                start=(c == 0), stop=(c == NDC - 1))
                        nc.any.tensor_copy(o_sb[:, ob * SB:(ob + 1) * SB], ps[:])
                    nc.sync.dma_start(out_r[:, sc, :], o_sb[:])

    nc.compile()
    return nc


def _host_inputs(x, W_Q, W_K, W_V, W_O, token_positions):
    """Build per-core input maps (all layout/permute work on host)."""
    pos = np.asarray(token_positions).reshape(-1).astype(np.float64)  # (S,)
    i = np.arange(DK // 2, dtype=np.float64)
    freqs = 1.0 / (THETA ** (2.0 * i / DK))          # (32,)
    ang = pos[None, :] * freqs[:, None]              # (32, S)
    cosR = np.tile(np.cos(ang), (4, 1)).astype(np.float32)   # (128, S)
    sinR = np.tile(np.sin(ang), (4, 1)).astype(np.float32)

    import ml_dtypes
    kk = np.arange(128)
    tri = (kk[:, None] <= kk[None, :]).astype(np.float32)    # 1 where k <= q
    masken = np.concatenate([tri, tri], axis=1).astype(ml_dtypes.bfloat16)

    swp = np.zeros((128, 128), dtype=np.float32)  # cast to bf16 below
    for g in (0, 64):
        for j in range(32):
            swp[g + 32 + j, g + j] = -1.0      # out[E] += -t2[O]
            swp[g + j, g + 32 + j] = 1.0       # out[O] += +t2[E]

    # row permutation for one head's 64 dims -> [evens(32) | odds(32)]
    eo = np.concatenate([np.arange(0, DK, 2), np.arange(1, DK, 2)])

    in_maps = []
    for c in range(NCORES):
        b = c // 2
        h0 = (c % 2) * HP
        r0 = h0 * DK
        rows = np.concatenate([lh * DK + eo for lh in range(HP)]) + r0  # (512,)
        wq = np.ascontiguousarray(W_Q[rows, :].T)   # (1024, 512)
        wk = np.ascontiguousarray(W_K[rows, :].T)
        wv = np.ascontiguousarray(W_V[r0:r0 + HP * DK, :].T)
        wo = np.ascontiguousarray(W_O[:, r0:r0 + HP * DK].T)  # (512, 1024)
        xT = np.ascontiguousarray(x[b].T)           # (1024, 2048)
        in_maps.append({
            "xT": xT.astype(ml_dtypes.bfloat16),
            "wqT": wq.astype(ml_dtypes.bfloat16),
            "wkT": wk.astype(ml_dtypes.bfloat16),
            "wvT": wv.astype(ml_dtypes.bfloat16),
            "woT": wo.astype(ml_dtypes.bfloat16),
            "cosR": cosR, "sinR": sinR,
            "swp": swp.astype(ml_dtypes.bfloat16),
            "masken": masken,
            "onesb": np.ones((128, 128), dtype=np.float32).astype(ml_dtypes.bfloat16),
        })
    return in_maps


class _Runner:
    """Persistent jitted SPMD executor (bass2jax PJRT path)."""

    def __init__(self, nc):
        import jax
        import numpy as _np
        from jax.sharding import Mesh, PartitionSpec
        from jax.experimental.shard_map import shard_map
        import concourse.mybir as mybir
        from concourse.bass2jax import (_bass_exec_p, partition_id_tensor,
                                        install_neuronx_cc_hook)
        install_neuronx_cc_hook()
        self.jax = jax
        in_names, out_names, out_avals, zero_outs = [], [], [], []
        partition_name = (nc.partition_id_tensor.name
                          if nc.partition_id_tensor else None)
        for alloc in nc.m.functions[0].allocations:
            if not isinstance(alloc, mybir.MemoryLocationSet):
                continue
            name = alloc.memorylocations[0].name
            if alloc.kind == "ExternalInput":
                if name != partition_name:
                    in_names.append(name)
            elif alloc.kind == "ExternalOutput":
                shape = tuple(alloc.tensor_shape)
                dtype = mybir.dt.np(alloc.dtype)
                out_names.append(name)
                out_avals.append(jax.core.ShapedArray(shape, dtype))
                zero_outs.append(_np.zeros(shape, dtype))
        self.in_names, self.out_names = in_names, out_names
        self.out_avals, self.zero_outs = out_avals, zero_outs
        n_params, n_outs = len(in_names), len(out_avals)
        all_in = in_names + out_names
        if partition_name is not None:
            all_in.append(partition_name)

        def _body(*args):
            operands = list(args)
            if partition_name is not None:
                operands.append(partition_id_tensor())
            return tuple(_bass_exec_p.bind(
                *operands, out_avals=tuple(out_avals), in_names=tuple(all_in),
                out_names=tuple(out_names), lowering_input_output_aliases=(),
                sim_require_finite=True, sim_require_nnan=True, nc=nc))

        devices = jax.devices()[:NCORES]
        mesh = Mesh(_np.asarray(devices), ("core",))
        self.fn = jax.jit(
            shard_map(_body, mesh=mesh,
                      in_specs=(PartitionSpec("core"),) * (n_params + n_outs),
                      out_specs=(PartitionSpec("core"),) * n_outs,
                      check_rep=False),
            keep_unused=True)

    def prepare(self, in_maps):
        np_ = np
        per_core = [[np_.asarray(m[name]) for name in self.in_names]
                    for m in in_maps]
        self._dev_in = [
            self.jax.device_put(np_.concatenate(
                [per_core[c][i] for c in range(NCORES)], axis=0))
            for i in range(len(self.in_names))]
        self._dev_zeros = [
            self.jax.device_put(np_.zeros((NCORES * z.shape[0], *z.shape[1:]),
                                          z.dtype))
            for z in self.zero_outs]

    def run(self):
        outs = self.fn(*self._dev_in, *self._dev_zeros)
        self.jax.block_until_ready(outs)
        return outs

    def results(self, outs):
        res = []
        for c in range(NCORES):
            res.append({
                name: np.asarray(outs[i]).reshape(
                    NCORES, *self.out_avals[i].shape)[c]
                for i, name in enumerate(self.out_names)})
        return res


def _get_runner(repeat=1):
    key = repeat
    if key not in _BUILD_CACHE:
        _BUILD_CACHE[key] = _Runner(_build(repeat))
    return _BUILD_CACHE[key]


def kernel(x, W_Q, W_K, W_V, W_O, token_positions):
    x = np.asarray(x, dtype=np.float32)
    W_Q = np.asarray(W_Q, dtype=np.float32)
    W_K = np.asarray(W_K, dtype=np.float32)
    W_V = np.asarray(W_V, dtype=np.float32)
    W_O = np.asarray(W_O, dtype=np.float32)
    r = _get_runner()
    r.prepare(_host_inputs(x, W_Q, W_K, W_V, W_O, token_positions))
    res = r.results(r.run())
    out = np.empty((B, S, D), dtype=np.float32)
    for b in range(B):
        out[b] = (res[2 * b]["out"].astype(np.float32)
                  + res[2 * b + 1]["out"].astype(np.float32))
    return out


# revision 10
# speedup vs baseline: 1.6476x; 1.6476x over previous
"""Causal multi-head self-attention (B=4, S=2048, D=1024, H=16, RoPE) on 8 NeuronCores.

Sharding: core c handles batch b = c // 2 and heads [8*(c%2), 8*(c%2)+8).
Each core computes its 8 heads' attention plus the partial W_O projection
(columns owned by its heads); host sums the two partials per batch.

v2: single causal-pipelined loop over s-blocks — for each 512-wide block sb:
  phase1(sb): QKV projections + RoPE for block sb
  attn(qb=sb): all 8 heads attend queries sb against keys 0..sb (available!)
  phase3(sb): W_O partial projection for block sb
This keeps the PE warm (no multi-us idle gaps -> no HAM re-throttle) and
overlaps the ACT-bound softmax with PE-bound projection work.
Softmax denominators: ones-column in V -> pv row 64; reciprocal_approx_fast;
gpsimd partition_broadcast; DVE multiply straight out of PSUM (no un copies,
no broadcast matmuls). Causal mask: DVE 0/1-mask multiply on exp output
(replaces PE bias matmuls).
"""
import math
import os
from contextlib import ExitStack, nullcontext

import numpy as np

B, S, D, H, DK = 4, 2048, 1024, 16, 64
HP = 8            # heads per core
NCORES = 8
THETA = 10000.0
SB = 512          # s-block width
NSB = S // SB     # 4
NIC = D // 128    # 8 in-chunks
NDC = (HP * DK) // 128   # 4 dk-chunks (local) = head pairs
NKC = S // 128    # 16 k-chunks

_BUILD_CACHE = {}


def _build(repeat=1):
    import concourse.tile as tile
    from concourse import bacc, mybir

    F32 = mybir.dt.float32
    BF16 = mybir.dt.bfloat16
    EXP = mybir.ActivationFunctionType.Exp
    MULT = mybir.AluOpType.mult
    ADD = mybir.AluOpType.add

    nc = bacc.Bacc("TRN2", target_bir_lowering=False, debug=False,
                   num_devices=NCORES)
    xT_d = nc.declare_dram_parameter("xT", [D, S], BF16, isOutput=False)
    wqT_d = nc.declare_dram_parameter("wqT", [D, HP * DK], BF16, isOutput=False)
    wkT_d = nc.declare_dram_parameter("wkT", [D, HP * DK], BF16, isOutput=False)
    wvT_d = nc.declare_dram_parameter("wvT", [D, HP * DK], BF16, isOutput=False)
    woT_d = nc.declare_dram_parameter("woT", [HP * DK, D], BF16, isOutput=False)
    cos_d = nc.declare_dram_parameter("cosR", [128, S], F32, isOutput=False)
    sin_d = nc.declare_dram_parameter("sinR", [128, S], F32, isOutput=False)
    swp_d = nc.declare_dram_parameter("swp", [128, 128], BF16, isOutput=False)
    mask_d = nc.declare_dram_parameter("masken", [128, 2 * 128], BF16, isOutput=False)
    onesb_d = nc.declare_dram_parameter("onesb", [128, 128], BF16, isOutput=False)
    out_d = nc.declare_dram_parameter("out", [S, D], BF16, isOutput=True)

    xT_r = xT_d.rearrange("(ic p) (sb s) -> p ic sb s", p=128, s=SB)
    out_r = out_d.rearrange("(sc p) o -> p sc o", p=128)

    with tile.TileContext(nc) as tc, ExitStack() as octx:
        # long-lived tensors
        glob = octx.enter_context(tc.tile_pool(name="glob", bufs=1))
        QT = glob.tile([128, NDC, S], BF16, tag="QT", name="QT")
        KT = glob.tile([128, NDC, S], BF16, tag="KT", name="KT")
        V = glob.tile([128, NKC, HP, DK + 1], BF16, tag="V", name="V")
        AO = glob.tile([128, NDC, S], BF16, tag="AO", name="AO")
        cosR = glob.tile([128, S], F32, tag="cosR", name="cosR")
        sinR = glob.tile([128, S], F32, tag="sinR", name="sinR")
        swp = glob.tile([128, 128], BF16, tag="swp", name="swp")
        maskt = glob.tile([128, 2, 128], BF16, tag="maskt", name="maskt")
        wq_sb = glob.tile([128, NIC, HP * DK], BF16, tag="wq", name="wq_sb")
        wk_sb = glob.tile([128, NIC, HP * DK], BF16, tag="wk", name="wk_sb")
        wv_sb = glob.tile([128, NIC, HP * DK], BF16, tag="wv", name="wv_sb")
        wo_sb = glob.tile([128, NDC, D], BF16, tag="wo", name="wo_sb")
        nc.sync.dma_start(cosR[:], cos_d[:])
        nc.sync.dma_start(sinR[:], sin_d[:])
        nc.sync.dma_start(swp[:], swp_d[:])
        nc.sync.dma_start(maskt[:], mask_d.rearrange("p (a q) -> p a q", a=2))
        nc.sync.dma_start(wq_sb[:], wqT_d.rearrange("(ic p) m -> p ic m", p=128))
        nc.sync.dma_start(wk_sb[:], wkT_d.rearrange("(ic p) m -> p ic m", p=128))
        nc.sync.dma_start(wv_sb[:], wvT_d.rearrange("(ic p) m -> p ic m", p=128))
        nc.sync.dma_start(wo_sb[:], woT_d.rearrange("(c p) o -> p c o", p=128))
        nc.sync.dma_start(
            V[:, :, :, DK:DK + 1],
            onesb_d.rearrange("p (a b c) -> p a b c", a=NKC, b=HP))

        xpool = octx.enter_context(tc.tile_pool(name="xpool", bufs=2))
        rpool = octx.enter_context(tc.tile_pool(name="rope", bufs=3))
        epool = octx.enter_context(tc.tile_pool(name="epool", bufs=int(os.environ.get("ETBUFS", "6"))))
        npool = octx.enter_context(tc.tile_pool(name="npool", bufs=2))
        opool = octx.enter_context(tc.tile_pool(name="opool", bufs=2))
        gps = octx.enter_context(tc.tile_pool(name="gps", bufs=int(os.environ.get("GBUFS", "2")), space="PSUM"))
        sps = octx.enter_context(tc.tile_pool(name="sps", bufs=2, space="PSUM"))
        pvps = octx.enter_context(tc.tile_pool(name="pvps", bufs=1, space="PSUM"))

        loop_cm = tc.For_i(0, repeat, 1) if repeat > 1 else nullcontext()
        with loop_cm:
            for sb in range(NSB):
                ssl = slice(sb * SB, (sb + 1) * SB)
                # ---------------- phase 1: projections + RoPE for block sb ----
                x_sb = xpool.tile([128, NIC, SB], BF16, tag="x", name=f"x_{sb}")
                nc.sync.dma_start(x_sb[:], xT_r[:, :, sb, :])
                for w_sb, OT in ((wq_sb, QT), (wk_sb, KT)):
                    for c in range(NDC):
                        ps = gps.tile([128, SB], F32, tag="g", name=f"ps_{sb}_{c}")
                        for ic in range(NIC):
                            nc.tensor.matmul(
                                ps[:], w_sb[:, ic, c * 128:(c + 1) * 128],
                                x_sb[:, ic, :],
                                start=(ic == 0), stop=(ic == NIC - 1))
                        t1 = rpool.tile([128, SB], F32, tag="t1", name="t1")
                        nc.vector.tensor_tensor(t1[:], ps[:], cosR[:, ssl], MULT)
                        t2 = rpool.tile([128, SB], BF16, tag="t2", name="t2")
                        nc.vector.tensor_tensor(t2[:], ps[:], sinR[:, ssl], MULT)
                        t2s = gps.tile([128, SB], F32, tag="g", name="t2s")
                        nc.tensor.matmul(t2s[:], swp[:], t2[:],
                                         start=True, stop=True)
                        nc.vector.tensor_tensor(OT[:, c, ssl], t1[:], t2s[:], ADD)
                # V (natural layout), psum partition dim = s-chunk
                for sc4 in range(SB // 128):
                    sc = sb * 4 + sc4
                    ps = gps.tile([128, HP * DK], F32, tag="g", name=f"psv_{sc}")
                    for ic in range(NIC):
                        nc.tensor.matmul(
                            ps[:], x_sb[:, ic, sc4 * 128:(sc4 + 1) * 128],
                            wv_sb[:, ic, :],
                            start=(ic == 0), stop=(ic == NIC - 1))
                    nc.vector.tensor_copy(
                        V[:, sc, :, 0:DK],
                        ps.rearrange("p (h v) -> p h v", h=HP))

                # ---------------- attention: queries block sb ----------------
                qb = sb
                nch = 4 * qb + 4
                qsl = slice(qb * SB, (qb + 1) * SB)
                for hp in range(NDC):
                    pv_a = pvps.tile([DK + 1, SB], F32, tag="pv_a", name=f"pva_{hp}_{qb}")
                    pv_b = pvps.tile([DK + 1, SB], F32, tag="pv_b", name=f"pvb_{hp}_{qb}")
                    for kc in range(nch):
                        ksl = slice(kc * 128, (kc + 1) * 128)
                        j = kc - 4 * qb
                        c0 = 128 * max(j, 0)
                        qsl0 = slice(qb * SB + c0, (qb + 1) * SB)
                        ps = sps.tile([128, 2, SB], F32, tag="s",
                                      name=f"pss_{hp}_{qb}_{kc}")
                        nc.tensor.matmul(
                            ps[:, 0, c0:], KT[0:64, hp, ksl], QT[0:64, hp, qsl0],
                            start=True, stop=True, tile_position=(0, 0))
                        nc.tensor.matmul(
                            ps[:, 1, c0:], KT[64:128, hp, ksl], QT[64:128, hp, qsl0],
                            start=True, stop=True, tile_position=(64, 0))
                        et = epool.tile([128, 2, SB], BF16, tag="et",
                                        name=f"et_{hp}_{qb}_{kc}")
                        nc.scalar.activation(et[:, :, c0:], ps[:, :, c0:],
                                             EXP, scale=1.0 / math.sqrt(DK))
                        if j >= 0:  # causal mask: 0/1 multiply on the diag block
                            nc.vector.tensor_tensor(
                                et[:, :, c0:c0 + 128], et[:, :, c0:c0 + 128],
                                maskt[:], MULT)
                        for x_, pv in ((0, pv_a), (1, pv_b)):
                            nc.tensor.matmul(
                                pv[:, c0:], V[:, kc, 2 * hp + x_, :],
                                et[:, x_, c0:],
                                start=(kc == 0), stop=(kc == nch - 1))
                    # normalize tail: recip of denominator row, broadcast, scale
                    dta = npool.tile([1, SB], F32, tag="dta", name="dta")
                    dtb = npool.tile([1, SB], F32, tag="dtb", name="dtb")
                    nc.vector.tensor_copy(dta[:], pv_a[DK:DK + 1, :])
                    nc.vector.tensor_copy(dtb[:], pv_b[DK:DK + 1, :])
                    ra = npool.tile([1, SB], F32, tag="ra", name="ra")
                    rb = npool.tile([1, SB], F32, tag="rb", name="rb")
                    nc.vector.reciprocal_approx_fast(out=ra[:], in_=dta[:])
                    nc.vector.reciprocal_approx_fast(out=rb[:], in_=dtb[:])
                    rbca = npool.tile([64, SB], F32, tag="rbca", name="rbca")
                    rbcb = npool.tile([64, SB], F32, tag="rbcb", name="rbcb")
                    nc.gpsimd.partition_broadcast(rbca[:], ra[:])
                    nc.gpsimd.partition_broadcast(rbcb[:], rb[:])
                    nc.vector.tensor_tensor(AO[0:64, hp, qsl], pv_a[0:DK, :],
                                            rbca[:], MULT)
                    nc.vector.tensor_tensor(AO[64:128, hp, qsl], pv_b[0:DK, :],
                                            rbcb[:], MULT)

                # ---------------- phase 3: W_O partials for block sb ----------
                for sc4 in range(SB // 128):
                    sc = sb * 4 + sc4
                    o_sb = opool.tile([128, D], BF16, tag="o", name=f"o_{sc}")
                    for ob in range(2):
                        ps = gps.tile([128, SB], F32, tag="g", name=f"ps3_{sc}_{ob}")
                        for c in range(NDC):
                            nc.tensor.matmul(
                                ps[:], AO[:, c, sc * 128:(sc + 1) * 128],
                                wo_sb[:, c, ob * SB:(ob + 1) * SB],
                                start=(c == 0), stop=(c == NDC - 1))
                        nc.vector.tensor_copy(o_sb[:, ob * SB:(ob + 1) * SB], ps[:])
                    nc.sync.dma_start(out_r[:, sc, :], o_sb[:])

    nc.compile()
    return nc


def _host_inputs(x, W_Q, W_K, W_V, W_O, token_positions):
    """Build per-core input maps (all layout/permute work on host)."""
    pos = np.asarray(token_positions).reshape(-1).astype(np.float64)  # (S,)
    i = np.arange(DK // 2, dtype=np.float64)
    freqs = 1.0 / (THETA ** (2.0 * i / DK))          # (32,)
    ang = pos[None, :] * freqs[:, None]              # (32, S)
    cosR = np.tile(np.cos(ang), (4, 1)).astype(np.float32)   # (128, S)
    sinR = np.tile(np.sin(ang), (4, 1)).astype(np.float32)

    import ml_dtypes
    kk = np.arange(128)
    tri = (kk[:, None] <= kk[None, :]).astype(np.float32)    # 1 where k <= q
    masken = np.concatenate([tri, tri], axis=1).astype(ml_dtypes.bfloat16)

    swp = np.zeros((128, 128), dtype=np.float32)  # cast to bf16 below
    for g in (0, 64):
        for j in range(32):
            swp[g + 32 + j, g + j] = -1.0      # out[E] += -t2[O]
            swp[g + j, g + 32 + j] = 1.0       # out[O] += +t2[E]

    # row permutation for one head's 64 dims -> [evens(32) | odds(32)]
    eo = np.concatenate([np.arange(0, DK, 2), np.arange(1, DK, 2)])

    in_maps = []
    for c in range(NCORES):
        b = c // 2
        h0 = (c % 2) * HP
        r0 = h0 * DK
        rows = np.concatenate([lh * DK + eo for lh in range(HP)]) + r0  # (512,)
        wq = np.ascontiguousarray(W_Q[rows, :].T)   # (1024, 512)
        wk = np.ascontiguousarray(W_K[rows, :].T)
        wv = np.ascontiguousarray(W_V[r0:r0 + HP * DK, :].T)
        wo = np.ascontiguousarray(W_O[:, r0:r0 + HP * DK].T)  # (512, 1024)
        xT = np.ascontiguousarray(x[b].T)           # (1024, 2048)
        in_maps.append({
            "xT": xT.astype(ml_dtypes.bfloat16),
            "wqT": wq.astype(ml_dtypes.bfloat16),
            "wkT": wk.astype(ml_dtypes.bfloat16),
            "wvT": wv.astype(ml_dtypes.bfloat16),
            "woT": wo.astype(ml_dtypes.bfloat16),
            "cosR": cosR, "sinR": sinR,
            "swp": swp.astype(ml_dtypes.bfloat16),
            "masken": masken,
            "onesb": np.ones((128, 128), dtype=np.float32).astype(ml_dtypes.bfloat16),
        })
    return in_maps


class _Runner:
    """Persistent jitted SPMD executor (bass2jax PJRT path)."""

    def __init__(self, nc):
        import jax
        import numpy as _np
        from jax.sharding import Mesh, PartitionSpec
        from jax.experimental.shard_map import shard_map
        import concourse.mybir as mybir
        from concourse.bass2jax import (_bass_exec_p, partition_id_tensor,
                                        install_neuronx_cc_hook)
        install_neuronx_cc_hook()
        self.jax = jax
        self.nc = nc
        in_names, out_names, out_avals, zero_outs = [], [], [], []
        partition_name = (nc.partition_id_tensor.name
                          if nc.partition_id_tensor else None)
        for alloc in nc.m.functions[0].allocations:
            if not isinstance(alloc, mybir.MemoryLocationSet):
                continue
            name = alloc.memorylocations[0].name
            if alloc.kind == "ExternalInput":
                if name != partition_name:
                    in_names.append(name)
            elif alloc.kind == "ExternalOutput":
                shape = tuple(alloc.tensor_shape)
                dtype = mybir.dt.np(alloc.dtype)
                out_names.append(name)
                out_avals.append(jax.core.ShapedArray(shape, dtype))
                zero_outs.append(_np.zeros(shape, dtype))
        self.in_names, self.out_names = in_names, out_names
        self.out_avals, self.zero_outs = out_avals, zero_outs
        n_params, n_outs = len(in_names), len(out_avals)
        all_in = in_names + out_names
        if partition_name is not None:
            all_in.append(partition_name)

        def _body(*args):
            operands = list(args)
            if partition_name is not None:
                operands.append(partition_id_tensor())
            return tuple(_bass_exec_p.bind(
                *operands, out_avals=tuple(out_avals), in_names=tuple(all_in),
                out_names=tuple(out_names), lowering_input_output_aliases=(),
                sim_require_finite=True, sim_require_nnan=True, nc=nc))

        devices = jax.devices()[:NCORES]
        mesh = Mesh(_np.asarray(devices), ("core",))
        self.fn = jax.jit(
            shard_map(_body, mesh=mesh,
                      in_specs=(PartitionSpec("core"),) * (n_params + n_outs),
                      out_specs=(PartitionSpec("core"),) * n_outs,
                      check_rep=False),
            keep_unused=True)

    def prepare(self, in_maps):
        np_ = np
        per_core = [[np_.asarray(m[name]) for name in self.in_names]
                    for m in in_maps]
        self._dev_in = [
            self.jax.device_put(np_.concatenate(
                [per_core[c][i] for c in range(NCORES)], axis=0))
            for i in range(len(self.in_names))]
        self._dev_zeros = [
            self.jax.device_put(np_.zeros((NCORES * z.shape[0], *z.shape[1:]),
                                          z.dtype))
            for z in self.zero_outs]

    def run(self):
        outs = self.fn(*self._dev_in, *self._dev_zeros)
        self.jax.block_until_ready(outs)
        return outs

    def results(self, outs):
        res = []
        for c in range(NCORES):
            res.append({
                name: np.asarray(outs[i]).reshape(
                    NCORES, *self.out_avals[i].shape)[c]
                for i, name in enumerate(self.out_names)})
        return res


def _get_runner(repeat=1):
    key = repeat
    if key not in _BUILD_CACHE:
        _BUILD_CACHE[key] = _Runner(_build(repeat))
    return _BUILD_CACHE[key]


def kernel(x, W_Q, W_K, W_V, W_O, token_positions):
    x = np.asarray(x, dtype=np.float32)
    W_Q = np.asarray(W_Q, dtype=np.float32)
    W_K = np.asarray(W_K, dtype=np.float32)
    W_V = np.asarray(W_V, dtype=np.float32)
    W_O = np.asarray(W_O, dtype=np.float32)
    r = _get_runner()
    r.prepare(_host_inputs(x, W_Q, W_K, W_V, W_O, token_positions))
    res = r.results(r.run())
    out = np.empty((B, S, D), dtype=np.float32)
    for b in range(B):
        out[b] = (res[2 * b]["out"].astype(np.float32)
                  + res[2 * b + 1]["out"].astype(np.float32))
    return out


# revision 11
# speedup vs baseline: 1.7410x; 1.0567x over previous
"""Causal multi-head self-attention (B=4, S=2048, D=1024, H=16, RoPE) on 8 NeuronCores.

Sharding: core c handles batch b = c // 2 and heads [8*(c%2), 8*(c%2)+8).
Each core computes its 8 heads' attention plus the partial W_O projection
(columns owned by its heads); host sums the two partials per batch.

v2: single causal-pipelined loop over s-blocks — for each 512-wide block sb:
  phase1(sb): QKV projections + RoPE for block sb
  attn(qb=sb): all 8 heads attend queries sb against keys 0..sb (available!)
  phase3(sb): W_O partial projection for block sb
This keeps the PE warm (no multi-us idle gaps -> no HAM re-throttle) and
overlaps the ACT-bound softmax with PE-bound projection work.
Softmax denominators: ones-column in V -> pv row 64; reciprocal_approx_fast;
gpsimd partition_broadcast; DVE multiply straight out of PSUM (no un copies,
no broadcast matmuls). Causal mask: DVE 0/1-mask multiply on exp output
(replaces PE bias matmuls).
"""
import math
import os
from contextlib import ExitStack, nullcontext

import numpy as np

B, S, D, H, DK = 4, 2048, 1024, 16, 64
HP = 8            # heads per core
NCORES = 8
THETA = 10000.0
SB = 512          # s-block width
NSB = S // SB     # 4
NIC = D // 128    # 8 in-chunks
NDC = (HP * DK) // 128   # 4 dk-chunks (local) = head pairs
NKC = S // 128    # 16 k-chunks

_BUILD_CACHE = {}


def _build(repeat=1):
    import concourse.tile as tile
    from concourse import bacc, mybir

    F32 = mybir.dt.float32
    BF16 = mybir.dt.bfloat16
    EXP = mybir.ActivationFunctionType.Exp
    MULT = mybir.AluOpType.mult
    ADD = mybir.AluOpType.add

    nc = bacc.Bacc("TRN2", target_bir_lowering=False, debug=False,
                   num_devices=NCORES)
    xT_d = nc.declare_dram_parameter("xT", [D, S], BF16, isOutput=False)
    wqT_d = nc.declare_dram_parameter("wqT", [D, HP * DK], BF16, isOutput=False)
    wkT_d = nc.declare_dram_parameter("wkT", [D, HP * DK], BF16, isOutput=False)
    wvT_d = nc.declare_dram_parameter("wvT", [D, HP * DK], BF16, isOutput=False)
    woT_d = nc.declare_dram_parameter("woT", [HP * DK, D], BF16, isOutput=False)
    cos_d = nc.declare_dram_parameter("cosR", [128, S], F32, isOutput=False)
    sin_d = nc.declare_dram_parameter("sinR", [128, S], F32, isOutput=False)
    swp_d = nc.declare_dram_parameter("swp", [128, 128], BF16, isOutput=False)
    mask_d = nc.declare_dram_parameter("masken", [128, 2 * 128], BF16, isOutput=False)
    onesb_d = nc.declare_dram_parameter("onesb", [128, 128], BF16, isOutput=False)
    out_d = nc.declare_dram_parameter("out", [S, D], BF16, isOutput=True)

    xT_r = xT_d.rearrange("(ic p) (sb s) -> p ic sb s", p=128, s=SB)
    out_r = out_d.rearrange("(sc p) o -> p sc o", p=128)

    with tile.TileContext(nc) as tc, ExitStack() as octx:
        # long-lived tensors
        glob = octx.enter_context(tc.tile_pool(name="glob", bufs=1))
        QT = glob.tile([128, NDC, S], BF16, tag="QT", name="QT")
        KT = glob.tile([128, NDC, S], BF16, tag="KT", name="KT")
        V = glob.tile([128, NKC, HP, DK + 1], BF16, tag="V", name="V")
        AO = glob.tile([128, NDC, S], BF16, tag="AO", name="AO")
        cosR = glob.tile([128, S], F32, tag="cosR", name="cosR")
        sinR = glob.tile([128, S], F32, tag="sinR", name="sinR")
        swp = glob.tile([128, 128], BF16, tag="swp", name="swp")
        maskt = glob.tile([128, 2, 128], BF16, tag="maskt", name="maskt")
        wq_sb = glob.tile([128, NIC, HP * DK], BF16, tag="wq", name="wq_sb")
        wk_sb = glob.tile([128, NIC, HP * DK], BF16, tag="wk", name="wk_sb")
        wv_sb = glob.tile([128, NIC, HP * DK], BF16, tag="wv", name="wv_sb")
        wo_sb = glob.tile([128, NDC, D], BF16, tag="wo", name="wo_sb")
        nc.sync.dma_start(cosR[:], cos_d[:])
        nc.sync.dma_start(sinR[:], sin_d[:])
        nc.sync.dma_start(swp[:], swp_d[:])
        nc.sync.dma_start(maskt[:], mask_d.rearrange("p (a q) -> p a q", a=2))
        nc.sync.dma_start(wq_sb[:], wqT_d.rearrange("(ic p) m -> p ic m", p=128))
        nc.sync.dma_start(wk_sb[:], wkT_d.rearrange("(ic p) m -> p ic m", p=128))
        nc.sync.dma_start(wv_sb[:], wvT_d.rearrange("(ic p) m -> p ic m", p=128))
        nc.sync.dma_start(wo_sb[:], woT_d.rearrange("(c p) o -> p c o", p=128))
        nc.sync.dma_start(
            V[:, :, :, DK:DK + 1],
            onesb_d.rearrange("p (a b c) -> p a b c", a=NKC, b=HP))

        xpool = octx.enter_context(tc.tile_pool(name="xpool", bufs=2))
        rpool = octx.enter_context(tc.tile_pool(name="rope", bufs=3))
        epool = octx.enter_context(tc.tile_pool(name="epool", bufs=int(os.environ.get("ETBUFS", "6"))))
        npool = octx.enter_context(tc.tile_pool(name="npool", bufs=2))
        opool = octx.enter_context(tc.tile_pool(name="opool", bufs=2))
        gps = octx.enter_context(tc.tile_pool(name="gps", bufs=int(os.environ.get("GBUFS", "2")), space="PSUM"))
        sps = octx.enter_context(tc.tile_pool(name="sps", bufs=2, space="PSUM"))
        pvps = octx.enter_context(tc.tile_pool(name="pvps", bufs=1, space="PSUM"))

        loop_cm = tc.For_i(0, repeat, 1) if repeat > 1 else nullcontext()
        with loop_cm:
            for sb in range(NSB):
                ssl = slice(sb * SB, (sb + 1) * SB)
                # ---------------- phase 1: projections + RoPE for block sb ----
                x_sb = xpool.tile([128, NIC, SB], BF16, tag="x", name=f"x_{sb}")
                nc.sync.dma_start(x_sb[:], xT_r[:, :, sb, :])
                # V first (needed by every head's pv matmuls)
                for sc4 in range(SB // 128):
                    sc = sb * 4 + sc4
                    ps = gps.tile([128, HP * DK], F32, tag="g", name=f"psv_{sc}")
                    for ic in range(NIC):
                        nc.tensor.matmul(
                            ps[:], x_sb[:, ic, sc4 * 128:(sc4 + 1) * 128],
                            wv_sb[:, ic, :],
                            start=(ic == 0), stop=(ic == NIC - 1))
                    nc.vector.tensor_copy(
                        V[:, sc, :, 0:DK],
                        ps.rearrange("p (h v) -> p h v", h=HP))

                # per head-pair: Q + K projection chunk, then that pair's
                # attention — starts the ACT-bound softmax stream early
                qb = sb
                nch = 4 * qb + 4
                qsl = slice(qb * SB, (qb + 1) * SB)
                for hp in range(NDC):
                    c = hp
                    for w_sb, OT in ((wq_sb, QT), (wk_sb, KT)):
                        ps = gps.tile([128, SB], F32, tag="g", name=f"ps_{sb}_{c}")
                        for ic in range(NIC):
                            nc.tensor.matmul(
                                ps[:], w_sb[:, ic, c * 128:(c + 1) * 128],
                                x_sb[:, ic, :],
                                start=(ic == 0), stop=(ic == NIC - 1))
                        t2 = rpool.tile([128, SB], BF16, tag="t2", name="t2")
                        nc.vector.tensor_tensor(t2[:], ps[:], sinR[:, ssl], MULT)
                        t1 = rpool.tile([128, SB], F32, tag="t1", name="t1")
                        nc.vector.tensor_tensor(t1[:], ps[:], cosR[:, ssl], MULT)
                        t2s = gps.tile([128, SB], F32, tag="g", name="t2s")
                        nc.tensor.matmul(t2s[:], swp[:], t2[:],
                                         start=True, stop=True)
                        nc.vector.tensor_tensor(OT[:, c, ssl], t1[:], t2s[:], ADD)
                    pv_a = pvps.tile([DK + 1, SB], F32, tag="pv_a", name=f"pva_{hp}_{qb}")
                    pv_b = pvps.tile([DK + 1, SB], F32, tag="pv_b", name=f"pvb_{hp}_{qb}")
                    for kc in range(nch):
                        ksl = slice(kc * 128, (kc + 1) * 128)
                        j = kc - 4 * qb
                        c0 = 128 * max(j, 0)
                        qsl0 = slice(qb * SB + c0, (qb + 1) * SB)
                        ps = sps.tile([128, 2, SB], F32, tag="s",
                                      name=f"pss_{hp}_{qb}_{kc}")
                        nc.tensor.matmul(
                            ps[:, 0, c0:], KT[0:64, hp, ksl], QT[0:64, hp, qsl0],
                            start=True, stop=True, tile_position=(0, 0))
                        nc.tensor.matmul(
                            ps[:, 1, c0:], KT[64:128, hp, ksl], QT[64:128, hp, qsl0],
                            start=True, stop=True, tile_position=(64, 0))
                        et = epool.tile([128, 2, SB], BF16, tag="et",
                                        name=f"et_{hp}_{qb}_{kc}")
                        nc.scalar.activation(et[:, :, c0:], ps[:, :, c0:],
                                             EXP, scale=1.0 / math.sqrt(DK))
                        if j >= 0:  # causal mask: 0/1 multiply on the diag block
                            nc.vector.tensor_tensor(
                                et[:, :, c0:c0 + 128], et[:, :, c0:c0 + 128],
                                maskt[:], MULT)
                        for x_, pv in ((0, pv_a), (1, pv_b)):
                            nc.tensor.matmul(
                                pv[:, c0:], V[:, kc, 2 * hp + x_, :],
                                et[:, x_, c0:],
                                start=(kc == 0), stop=(kc == nch - 1))
                    # normalize tail: recip of denominator row, broadcast, scale
                    dta = npool.tile([1, SB], F32, tag="dta", name="dta")
                    dtb = npool.tile([1, SB], F32, tag="dtb", name="dtb")
                    nc.vector.tensor_copy(dta[:], pv_a[DK:DK + 1, :])
                    nc.vector.tensor_copy(dtb[:], pv_b[DK:DK + 1, :])
                    ra = npool.tile([1, SB], F32, tag="ra", name="ra")
                    rb = npool.tile([1, SB], F32, tag="rb", name="rb")
                    nc.vector.reciprocal_approx_fast(out=ra[:], in_=dta[:])
                    nc.vector.reciprocal_approx_fast(out=rb[:], in_=dtb[:])
                    rbca = npool.tile([64, SB], F32, tag="rbca", name="rbca")
                    rbcb = npool.tile([64, SB], F32, tag="rbcb", name="rbcb")
                    nc.gpsimd.partition_broadcast(rbca[:], ra[:])
                    nc.gpsimd.partition_broadcast(rbcb[:], rb[:])
                    nc.vector.tensor_tensor(AO[0:64, hp, qsl], pv_a[0:DK, :],
                                            rbca[:], MULT)
                    nc.vector.tensor_tensor(AO[64:128, hp, qsl], pv_b[0:DK, :],
                                            rbcb[:], MULT)

                # ---------------- phase 3: W_O partials for block sb ----------
                for sc4 in range(SB // 128):
                    sc = sb * 4 + sc4
                    o_sb = opool.tile([128, D], BF16, tag="o", name=f"o_{sc}")
                    for ob in range(2):
                        ps = gps.tile([128, SB], F32, tag="g", name=f"ps3_{sc}_{ob}")
                        for c in range(NDC):
                            nc.tensor.matmul(
                                ps[:], AO[:, c, sc * 128:(sc + 1) * 128],
                                wo_sb[:, c, ob * SB:(ob + 1) * SB],
                                start=(c == 0), stop=(c == NDC - 1))
                        nc.vector.tensor_copy(o_sb[:, ob * SB:(ob + 1) * SB], ps[:])
                    nc.sync.dma_start(out_r[:, sc, :], o_sb[:])

    nc.compile()
    return nc


def _host_inputs(x, W_Q, W_K, W_V, W_O, token_positions):
    """Build per-core input maps (all layout/permute work on host)."""
    pos = np.asarray(token_positions).reshape(-1).astype(np.float64)  # (S,)
    i = np.arange(DK // 2, dtype=np.float64)
    freqs = 1.0 / (THETA ** (2.0 * i / DK))          # (32,)
    ang = pos[None, :] * freqs[:, None]              # (32, S)
    cosR = np.tile(np.cos(ang), (4, 1)).astype(np.float32)   # (128, S)
    sinR = np.tile(np.sin(ang), (4, 1)).astype(np.float32)

    import ml_dtypes
    kk = np.arange(128)
    tri = (kk[:, None] <= kk[None, :]).astype(np.float32)    # 1 where k <= q
    masken = np.concatenate([tri, tri], axis=1).astype(ml_dtypes.bfloat16)

    swp = np.zeros((128, 128), dtype=np.float32)  # cast to bf16 below
    for g in (0, 64):
        for j in range(32):
            swp[g + 32 + j, g + j] = -1.0      # out[E] += -t2[O]
            swp[g + j, g + 32 + j] = 1.0       # out[O] += +t2[E]

    # row permutation for one head's 64 dims -> [evens(32) | odds(32)]
    eo = np.concatenate([np.arange(0, DK, 2), np.arange(1, DK, 2)])

    in_maps = []
    for c in range(NCORES):
        b = c // 2
        h0 = (c % 2) * HP
        r0 = h0 * DK
        rows = np.concatenate([lh * DK + eo for lh in range(HP)]) + r0  # (512,)
        wq = np.ascontiguousarray(W_Q[rows, :].T)   # (1024, 512)
        wk = np.ascontiguousarray(W_K[rows, :].T)
        wv = np.ascontiguousarray(W_V[r0:r0 + HP * DK, :].T)
        wo = np.ascontiguousarray(W_O[:, r0:r0 + HP * DK].T)  # (512, 1024)
        xT = np.ascontiguousarray(x[b].T)           # (1024, 2048)
        in_maps.append({
            "xT": xT.astype(ml_dtypes.bfloat16),
            "wqT": wq.astype(ml_dtypes.bfloat16),
            "wkT": wk.astype(ml_dtypes.bfloat16),
            "wvT": wv.astype(ml_dtypes.bfloat16),
            "woT": wo.astype(ml_dtypes.bfloat16),
            "cosR": cosR, "sinR": sinR,
            "swp": swp.astype(ml_dtypes.bfloat16),
            "masken": masken,
            "onesb": np.ones((128, 128), dtype=np.float32).astype(ml_dtypes.bfloat16),
        })
    return in_maps


class _Runner:
    """Persistent jitted SPMD executor (bass2jax PJRT path)."""

    def __init__(self, nc):
        import jax
        import numpy as _np
        from jax.sharding import Mesh, PartitionSpec
        from jax.experimental.shard_map import shard_map
        import concourse.mybir as mybir
        from concourse.bass2jax import (_bass_exec_p, partition_id_tensor,
                                        install_neuronx_cc_hook)
        install_neuronx_cc_hook()
        self.jax = jax
        self.nc = nc
        in_names, out_names, out_avals, zero_outs = [], [], [], []
        partition_name = (nc.partition_id_tensor.name
                          if nc.partition_id_tensor else None)
        for alloc in nc.m.functions[0].allocations:
            if not isinstance(alloc, mybir.MemoryLocationSet):
                continue
            name = alloc.memorylocations[0].name
            if alloc.kind == "ExternalInput":
                if name != partition_name:
                    in_names.append(name)
            elif alloc.kind == "ExternalOutput":
                shape = tuple(alloc.tensor_shape)
                dtype = mybir.dt.np(alloc.dtype)
                out_names.append(name)
                out_avals.append(jax.core.ShapedArray(shape, dtype))
                zero_outs.append(_np.zeros(shape, dtype))
        self.in_names, self.out_names = in_names, out_names
        self.out_avals, self.zero_outs = out_avals, zero_outs
        n_params, n_outs = len(in_names), len(out_avals)
        all_in = in_names + out_names
        if partition_name is not None:
            all_in.append(partition_name)

        def _body(*args):
            operands = list(args)
            if partition_name is not None:
                operands.append(partition_id_tensor())
            return tuple(_bass_exec_p.bind(
                *operands, out_avals=tuple(out_avals), in_names=tuple(all_in),
                out_names=tuple(out_names), lowering_input_output_aliases=(),
                sim_require_finite=True, sim_require_nnan=True, nc=nc))

        devices = jax.devices()[:NCORES]
        mesh = Mesh(_np.asarray(devices), ("core",))
        self.fn = jax.jit(
            shard_map(_body, mesh=mesh,
                      in_specs=(PartitionSpec("core"),) * (n_params + n_outs),
                      out_specs=(PartitionSpec("core"),) * n_outs,
                      check_rep=False),
            keep_unused=True)

    def prepare(self, in_maps):
        np_ = np
        per_core = [[np_.asarray(m[name]) for name in self.in_names]
                    for m in in_maps]
        self._dev_in = [
            self.jax.device_put(np_.concatenate(
                [per_core[c][i] for c in range(NCORES)], axis=0))
            for i in range(len(self.in_names))]
        self._dev_zeros = [
            self.jax.device_put(np_.zeros((NCORES * z.shape[0], *z.shape[1:]),
                                          z.dtype))
            for z in self.zero_outs]

    def run(self):
        outs = self.fn(*self._dev_in, *self._dev_zeros)
        self.jax.block_until_ready(outs)
        return outs

    def results(self, outs):
        res = []
        for c in range(NCORES):
            res.append({
                name: np.asarray(outs[i]).reshape(
                    NCORES, *self.out_avals[i].shape)[c]
                for i, name in enumerate(self.out_names)})
        return res


def _get_runner(repeat=1):
    key = repeat
    if key not in _BUILD_CACHE:
        _BUILD_CACHE[key] = _Runner(_build(repeat))
    return _BUILD_CACHE[key]


def kernel(x, W_Q, W_K, W_V, W_O, token_positions):
    x = np.asarray(x, dtype=np.float32)
    W_Q = np.asarray(W_Q, dtype=np.float32)
    W_K = np.asarray(W_K, dtype=np.float32)
    W_V = np.asarray(W_V, dtype=np.float32)
    W_O = np.asarray(W_O, dtype=np.float32)
    r = _get_runner()
    r.prepare(_host_inputs(x, W_Q, W_K, W_V, W_O, token_positions))
    res = r.results(r.run())
    out = np.empty((B, S, D), dtype=np.float32)
    for b in range(B):
        out[b] = (res[2 * b]["out"].astype(np.float32)
                  + res[2 * b + 1]["out"].astype(np.float32))
    return out


# revision 15
# speedup vs baseline: 1.8568x; 1.0665x over previous
"""Causal multi-head self-attention (B=4, S=2048, D=1024, H=16, RoPE) on 8 NeuronCores.

Sharding: core c handles batch b = c // 2 and heads [8*(c%2), 8*(c%2)+8).
Each core computes its 8 heads' attention plus the partial W_O projection
(columns owned by its heads); host sums the two partials per batch.

v2: single causal-pipelined loop over s-blocks — for each 512-wide block sb:
  phase1(sb): QKV projections + RoPE for block sb
  attn(qb=sb): all 8 heads attend queries sb against keys 0..sb (available!)
  phase3(sb): W_O partial projection for block sb
This keeps the PE warm (no multi-us idle gaps -> no HAM re-throttle) and
overlaps the ACT-bound softmax with PE-bound projection work.
Softmax denominators: ones-column in V -> pv row 64; reciprocal_approx_fast;
gpsimd partition_broadcast; DVE multiply straight out of PSUM (no un copies,
no broadcast matmuls). Causal mask: DVE 0/1-mask multiply on exp output
(replaces PE bias matmuls).
"""
import math
import os
from contextlib import ExitStack, nullcontext

import numpy as np

B, S, D, H, DK = 4, 2048, 1024, 16, 64
HP = 8            # heads per core
NCORES = 8
THETA = 10000.0
SB = 512          # s-block width
NSB = S // SB     # 4
NIC = D // 128    # 8 in-chunks
NDC = (HP * DK) // 128   # 4 dk-chunks (local) = head pairs
NKC = S // 128    # 16 k-chunks

_BUILD_CACHE = {}


def _build(repeat=1):
    import concourse.tile as tile
    from concourse import bacc, mybir

    F32 = mybir.dt.float32
    BF16 = mybir.dt.bfloat16
    EXP = mybir.ActivationFunctionType.Exp
    MULT = mybir.AluOpType.mult
    ADD = mybir.AluOpType.add

    nc = bacc.Bacc("TRN2", target_bir_lowering=False, debug=False,
                   num_devices=NCORES)
    xT_d = nc.declare_dram_parameter("xT", [D, S], BF16, isOutput=False)
    wqT_d = nc.declare_dram_parameter("wqT", [D, HP * DK], BF16, isOutput=False)
    wkT_d = nc.declare_dram_parameter("wkT", [D, HP * DK], BF16, isOutput=False)
    wvT_d = nc.declare_dram_parameter("wvT", [D, HP * DK], BF16, isOutput=False)
    woT_d = nc.declare_dram_parameter("woT", [HP * DK, D], BF16, isOutput=False)
    cos_d = nc.declare_dram_parameter("cosR", [128, S], BF16, isOutput=False)
    sin_d = nc.declare_dram_parameter("sinR", [128, S], BF16, isOutput=False)
    swp_d = nc.declare_dram_parameter("swp", [128, 128], BF16, isOutput=False)
    mask_d = nc.declare_dram_parameter("masken", [128, 2 * 128], BF16, isOutput=False)
    onesb_d = nc.declare_dram_parameter("onesb", [128, 128], BF16, isOutput=False)
    out_d = nc.declare_dram_parameter("out", [S, D], BF16, isOutput=True)

    xT_r = xT_d.rearrange("(ic p) (sb s) -> p ic sb s", p=128, s=SB)
    out_r = out_d.rearrange("(sc p) o -> p sc o", p=128)

    with tile.TileContext(nc) as tc, ExitStack() as octx:
        # long-lived tensors
        glob = octx.enter_context(tc.tile_pool(name="glob", bufs=1))
        QT = glob.tile([128, NDC, S], BF16, tag="QT", name="QT")
        KT = glob.tile([128, NDC, S], BF16, tag="KT", name="KT")
        V = glob.tile([128, NKC, HP, DK + 1], BF16, tag="V", name="V")
        AO = glob.tile([128, NDC, S], BF16, tag="AO", name="AO")
        cosR = glob.tile([128, S], BF16, tag="cosR", name="cosR")
        sinR = glob.tile([128, S], BF16, tag="sinR", name="sinR")
        swp = glob.tile([128, 128], BF16, tag="swp", name="swp")
        maskt = glob.tile([128, 2, 128], BF16, tag="maskt", name="maskt")
        wq_sb = glob.tile([128, NIC, HP * DK], BF16, tag="wq", name="wq_sb")
        wk_sb = glob.tile([128, NIC, HP * DK], BF16, tag="wk", name="wk_sb")
        wv_sb = glob.tile([128, NIC, HP * DK], BF16, tag="wv", name="wv_sb")
        wo_sb = glob.tile([128, NDC, D], BF16, tag="wo", name="wo_sb")

        xpool = octx.enter_context(tc.tile_pool(name="xpool", bufs=4))
        rpool = octx.enter_context(tc.tile_pool(name="rope", bufs=3))
        epool = octx.enter_context(tc.tile_pool(name="epool", bufs=int(os.environ.get("ETBUFS", "6"))))
        npool = octx.enter_context(tc.tile_pool(name="npool", bufs=2))
        opool = octx.enter_context(tc.tile_pool(name="opool", bufs=2))
        gps = octx.enter_context(tc.tile_pool(name="gps", bufs=int(os.environ.get("GBUFS", "2")), space="PSUM"))
        sps = octx.enter_context(tc.tile_pool(name="sps", bufs=2, space="PSUM"))
        pvps = octx.enter_context(tc.tile_pool(name="pvps", bufs=1, space="PSUM"))

        loop_cm = tc.For_i(0, repeat, 1) if repeat > 1 else nullcontext()
        with loop_cm:
            # input DMAs ordered so first compute (V(0), QK(0)) unblocks ASAP;
            # wo is not needed until the first phase-3 (~40us in)
            x_tiles = []
            for sb in range(NSB):
                x_sb = xpool.tile([128, NIC, SB], BF16, tag="x", name=f"x_{sb}")
                x_tiles.append(x_sb)
            nc.sync.dma_start(x_tiles[0][:], xT_r[:, :, 0, :])
            nc.sync.dma_start(wv_sb[:], wvT_d.rearrange("(ic p) m -> p ic m", p=128))
            nc.sync.dma_start(wq_sb[:], wqT_d.rearrange("(ic p) m -> p ic m", p=128))
            nc.sync.dma_start(wk_sb[:], wkT_d.rearrange("(ic p) m -> p ic m", p=128))
            nc.sync.dma_start(cosR[:], cos_d[:])
            nc.sync.dma_start(sinR[:], sin_d[:])
            nc.sync.dma_start(swp[:], swp_d[:])
            nc.sync.dma_start(maskt[:], mask_d.rearrange("p (a q) -> p a q", a=2))
            nc.sync.dma_start(
                V[:, :, :, DK:DK + 1],
                onesb_d.rearrange("p (a b c) -> p a b c", a=NKC, b=HP))
            for sb in range(1, NSB):
                nc.sync.dma_start(x_tiles[sb][:], xT_r[:, :, sb, :])
            nc.sync.dma_start(wo_sb[:], woT_d.rearrange("(c p) o -> p c o", p=128))
            for sb in range(NSB):
                ssl = slice(sb * SB, (sb + 1) * SB)
                # ---------------- phase 1: projections + RoPE for block sb ----
                x_sb = x_tiles[sb]
                # V first (needed by every head's pv matmuls)
                for sc4 in range(SB // 128):
                    sc = sb * 4 + sc4
                    ps = gps.tile([128, HP * DK], F32, tag="g", name=f"psv_{sc}")
                    for ic in range(NIC):
                        nc.tensor.matmul(
                            ps[:], x_sb[:, ic, sc4 * 128:(sc4 + 1) * 128],
                            wv_sb[:, ic, :],
                            start=(ic == 0), stop=(ic == NIC - 1))
                    nc.vector.tensor_copy(
                        V[:, sc, :, 0:DK],
                        ps.rearrange("p (h v) -> p h v", h=HP))

                # per head-pair: Q + K projection chunk, then that pair's
                # attention — starts the ACT-bound softmax stream early
                qb = sb
                nch = 4 * qb + 4
                qsl = slice(qb * SB, (qb + 1) * SB)
                for hp in range(NDC):
                    c = hp
                    for w_sb, OT in ((wq_sb, QT), (wk_sb, KT)):
                        ps = gps.tile([128, SB], F32, tag="g", name=f"ps_{sb}_{c}")
                        for ic in range(NIC):
                            nc.tensor.matmul(
                                ps[:], w_sb[:, ic, c * 128:(c + 1) * 128],
                                x_sb[:, ic, :],
                                start=(ic == 0), stop=(ic == NIC - 1))
                        t2 = rpool.tile([128, SB], BF16, tag="t2", name="t2")
                        nc.vector.tensor_tensor(t2[:], ps[:], sinR[:, ssl], MULT)
                        t1 = rpool.tile([128, SB], F32, tag="t1", name="t1")
                        nc.vector.tensor_tensor(t1[:], ps[:], cosR[:, ssl], MULT)
                        t2s = gps.tile([128, SB], F32, tag="g", name="t2s")
                        nc.tensor.matmul(t2s[:], swp[:], t2[:],
                                         start=True, stop=True)
                        nc.vector.tensor_tensor(OT[:, c, ssl], t1[:], t2s[:], ADD)
                    pv_a = pvps.tile([DK + 1, SB], F32, tag="pv_a", name=f"pva_{hp}_{qb}")
                    pv_b = pvps.tile([DK + 1, SB], F32, tag="pv_b", name=f"pvb_{hp}_{qb}")
                    for kc in range(nch):
                        ksl = slice(kc * 128, (kc + 1) * 128)
                        j = kc - 4 * qb
                        c0 = 128 * max(j, 0)
                        qsl0 = slice(qb * SB + c0, (qb + 1) * SB)
                        ps = sps.tile([128, 2, SB], F32, tag="s",
                                      name=f"pss_{hp}_{qb}_{kc}")
                        nc.tensor.matmul(
                            ps[:, 0, c0:], KT[0:64, hp, ksl], QT[0:64, hp, qsl0],
                            start=True, stop=True, tile_position=(0, 0))
                        nc.tensor.matmul(
                            ps[:, 1, c0:], KT[64:128, hp, ksl], QT[64:128, hp, qsl0],
                            start=True, stop=True, tile_position=(64, 0))
                        et = epool.tile([128, 2, SB], BF16, tag="et",
                                        name=f"et_{hp}_{qb}_{kc}")
                        nc.scalar.activation(et[:, :, c0:], ps[:, :, c0:],
                                             EXP, scale=1.0 / math.sqrt(DK))
                        if j >= 0:  # causal mask: 0/1 multiply on the diag block
                            nc.vector.tensor_tensor(
                                et[:, :, c0:c0 + 128], et[:, :, c0:c0 + 128],
                                maskt[:], MULT)
                        for x_, pv in ((0, pv_a), (1, pv_b)):
                            nc.tensor.matmul(
                                pv[:, c0:], V[:, kc, 2 * hp + x_, :],
                                et[:, x_, c0:],
                                start=(kc == 0), stop=(kc == nch - 1))
                    # normalize tail: recip of denominator row, broadcast, scale
                    dta = npool.tile([1, SB], F32, tag="dta", name="dta")
                    dtb = npool.tile([1, SB], F32, tag="dtb", name="dtb")
                    nc.vector.tensor_copy(dta[:], pv_a[DK:DK + 1, :])
                    nc.vector.tensor_copy(dtb[:], pv_b[DK:DK + 1, :])
                    ra = npool.tile([1, SB], F32, tag="ra", name="ra")
                    rb = npool.tile([1, SB], F32, tag="rb", name="rb")
                    nc.vector.reciprocal_approx_fast(out=ra[:], in_=dta[:])
                    nc.vector.reciprocal_approx_fast(out=rb[:], in_=dtb[:])
                    rbca = npool.tile([64, SB], F32, tag="rbca", name="rbca")
                    rbcb = npool.tile([64, SB], F32, tag="rbcb", name="rbcb")
                    nc.gpsimd.partition_broadcast(rbca[:], ra[:])
                    nc.gpsimd.partition_broadcast(rbcb[:], rb[:])
                    nc.vector.tensor_tensor(AO[0:64, hp, qsl], pv_a[0:DK, :],
                                            rbca[:], MULT)
                    nc.vector.tensor_tensor(AO[64:128, hp, qsl], pv_b[0:DK, :],
                                            rbcb[:], MULT)

                # ---------------- phase 3: W_O partials for block sb ----------
                for sc4 in range(SB // 128):
                    sc = sb * 4 + sc4
                    o_sb = opool.tile([128, D], BF16, tag="o", name=f"o_{sc}")
                    for ob in range(2):
                        ps = gps.tile([128, SB], F32, tag="g", name=f"ps3_{sc}_{ob}")
                        for c in range(NDC):
                            nc.tensor.matmul(
                                ps[:], AO[:, c, sc * 128:(sc + 1) * 128],
                                wo_sb[:, c, ob * SB:(ob + 1) * SB],
                                start=(c == 0), stop=(c == NDC - 1))
                        nc.vector.tensor_copy(o_sb[:, ob * SB:(ob + 1) * SB], ps[:])
                    nc.sync.dma_start(out_r[:, sc, :], o_sb[:])

    nc.compile()
    return nc


def _host_inputs(x, W_Q, W_K, W_V, W_O, token_positions):
    """Build per-core input maps (all layout/permute work on host)."""
    pos = np.asarray(token_positions).reshape(-1).astype(np.float64)  # (S,)
    i = np.arange(DK // 2, dtype=np.float64)
    freqs = 1.0 / (THETA ** (2.0 * i / DK))          # (32,)
    ang = pos[None, :] * freqs[:, None]              # (32, S)
    cosR = np.tile(np.cos(ang), (4, 1)).astype(np.float32)   # (128, S)
    sinR = np.tile(np.sin(ang), (4, 1)).astype(np.float32)

    import ml_dtypes
    kk = np.arange(128)
    tri = (kk[:, None] <= kk[None, :]).astype(np.float32)    # 1 where k <= q
    masken = np.concatenate([tri, tri], axis=1).astype(ml_dtypes.bfloat16)

    swp = np.zeros((128, 128), dtype=np.float32)  # cast to bf16 below
    for g in (0, 64):
        for j in range(32):
            swp[g + 32 + j, g + j] = -1.0      # out[E] += -t2[O]
            swp[g + j, g + 32 + j] = 1.0       # out[O] += +t2[E]

    # row permutation for one head's 64 dims -> [evens(32) | odds(32)]
    eo = np.concatenate([np.arange(0, DK, 2), np.arange(1, DK, 2)])

    in_maps = []
    for c in range(NCORES):
        b = c // 2
        h0 = (c % 2) * HP
        r0 = h0 * DK
        rows = np.concatenate([lh * DK + eo for lh in range(HP)]) + r0  # (512,)
        wq = np.ascontiguousarray(W_Q[rows, :].T)   # (1024, 512)
        wk = np.ascontiguousarray(W_K[rows, :].T)
        wv = np.ascontiguousarray(W_V[r0:r0 + HP * DK, :].T)
        wo = np.ascontiguousarray(W_O[:, r0:r0 + HP * DK].T)  # (512, 1024)
        xT = np.ascontiguousarray(x[b].T)           # (1024, 2048)
        in_maps.append({
            "xT": xT.astype(ml_dtypes.bfloat16),
            "wqT": wq.astype(ml_dtypes.bfloat16),
            "wkT": wk.astype(ml_dtypes.bfloat16),
            "wvT": wv.astype(ml_dtypes.bfloat16),
            "woT": wo.astype(ml_dtypes.bfloat16),
            "cosR": cosR.astype(ml_dtypes.bfloat16),
            "sinR": sinR.astype(ml_dtypes.bfloat16),
            "swp": swp.astype(ml_dtypes.bfloat16),
            "masken": masken,
            "onesb": np.ones((128, 128), dtype=np.float32).astype(ml_dtypes.bfloat16),
        })
    return in_maps


class _Runner:
    """Persistent jitted SPMD executor (bass2jax PJRT path)."""

    def __init__(self, nc):
        import jax
        import numpy as _np
        from jax.sharding import Mesh, PartitionSpec
        from jax.experimental.shard_map import shard_map
        import concourse.mybir as mybir
        from concourse.bass2jax import (_bass_exec_p, partition_id_tensor,
                                        install_neuronx_cc_hook)
        install_neuronx_cc_hook()
        self.jax = jax
        self.nc = nc
        in_names, out_names, out_avals, zero_outs = [], [], [], []
        partition_name = (nc.partition_id_tensor.name
                          if nc.partition_id_tensor else None)
        for alloc in nc.m.functions[0].allocations:
            if not isinstance(alloc, mybir.MemoryLocationSet):
                continue
            name = alloc.memorylocations[0].name
            if alloc.kind == "ExternalInput":
                if name != partition_name:
                    in_names.append(name)
            elif alloc.kind == "ExternalOutput":
                shape = tuple(alloc.tensor_shape)
                dtype = mybir.dt.np(alloc.dtype)
                out_names.append(name)
                out_avals.append(jax.core.ShapedArray(shape, dtype))
                zero_outs.append(_np.zeros(shape, dtype))
        self.in_names, self.out_names = in_names, out_names
        self.out_avals, self.zero_outs = out_avals, zero_outs
        n_params, n_outs = len(in_names), len(out_avals)
        all_in = in_names + out_names
        if partition_name is not None:
            all_in.append(partition_name)

        def _body(*args):
            operands = list(args)
            if partition_name is not None:
                operands.append(partition_id_tensor())
            return tuple(_bass_exec_p.bind(
                *operands, out_avals=tuple(out_avals), in_names=tuple(all_in),
                out_names=tuple(out_names), lowering_input_output_aliases=(),
                sim_require_finite=True, sim_require_nnan=True, nc=nc))

        devices = jax.devices()[:NCORES]
        mesh = Mesh(_np.asarray(devices), ("core",))
        self.fn = jax.jit(
            shard_map(_body, mesh=mesh,
                      in_specs=(PartitionSpec("core"),) * (n_params + n_outs),
                      out_specs=(PartitionSpec("core"),) * n_outs,
                      check_rep=False),
            keep_unused=True)

    def prepare(self, in_maps):
        np_ = np
        per_core = [[np_.asarray(m[name]) for name in self.in_names]
                    for m in in_maps]
        self._dev_in = [
            self.jax.device_put(np_.concatenate(
                [per_core[c][i] for c in range(NCORES)], axis=0))
            for i in range(len(self.in_names))]
        self._dev_zeros = [
            self.jax.device_put(np_.zeros((NCORES * z.shape[0], *z.shape[1:]),
                                          z.dtype))
            for z in self.zero_outs]

    def run(self):
        outs = self.fn(*self._dev_in, *self._dev_zeros)
        self.jax.block_until_ready(outs)
        return outs

    def results(self, outs):
        res = []
        for c in range(NCORES):
            res.append({
                name: np.asarray(outs[i]).reshape(
                    NCORES, *self.out_avals[i].shape)[c]
                for i, name in enumerate(self.out_names)})
        return res


def _get_runner(repeat=1):
    key = repeat
    if key not in _BUILD_CACHE:
        _BUILD_CACHE[key] = _Runner(_build(repeat))
    return _BUILD_CACHE[key]


def kernel(x, W_Q, W_K, W_V, W_O, token_positions):
    x = np.asarray(x, dtype=np.float32)
    W_Q = np.asarray(W_Q, dtype=np.float32)
    W_K = np.asarray(W_K, dtype=np.float32)
    W_V = np.asarray(W_V, dtype=np.float32)
    W_O = np.asarray(W_O, dtype=np.float32)
    r = _get_runner()
    r.prepare(_host_inputs(x, W_Q, W_K, W_V, W_O, token_positions))
    res = r.results(r.run())
    out = np.empty((B, S, D), dtype=np.float32)
    for b in range(B):
        out[b] = (res[2 * b]["out"].astype(np.float32)
                  + res[2 * b + 1]["out"].astype(np.float32))
    return out
